# revision 1
# baseline (speedup 1.0000x reference)
"""CARAFE content-aware upsampling on 8 Trainium2 NeuronCores (Bass/Tile).

Problem: features (4,128,64,64) f32, masks (4,25,128,128) f32
         -> out (4,128,128,128) f32
out[n,c,2h+a,2w+b] = sum_{i,j in 5x5} f[n,c,h+i-2,w+j-2] * m[n,5i+j,2h+a,2w+b]

Strategy (per core = one (n, h-half) shard):
  For each low-res row h we compute out[c, (a, wup)] (two upsampled rows,
  256 cols) as 5 PSUM-accumulated fp32r matmuls, one per kernel-row i:
     out += fT_row(h+i-2).T @ B_i
  where fT_row is the W-padded transposed feature row [w''(68), c(128)]
  (host-pretransposed) and B_i [w''(68), 256 cols] is a banded matrix
  holding the masks on diagonals.  Band columns are laid out (w, b, a) so
  each partition's band content is one contiguous 20-element (80 B) run;
  the matmul rhs reads it back as (a, wup) via a stride-2 inner AP.
  Bands are materialized by a per-job SBUF->SBUF diagonal-scatter DMA
  (dest AP steps +1 partition +4 elements) out of a bulk-loaded staging
  copy of the host-rearranged masks.  The band sparsity pattern is
  static, so the zero background is memset once and runs are overwritten
  in place; run overrun at the edges lands in 16-element pad gaps
  between the five band regions.
"""
import sys

if "/opt/trn_rl_repo" not in sys.path:
    sys.path.insert(0, "/opt/trn_rl_repo")

from contextlib import ExitStack

import numpy as np

import concourse.tile as tile
from concourse import bacc, mybir
from concourse.ap import AP
from concourse.bass_utils import run_bass_kernel_spmd

# ---- problem constants (hardcoded per harness contract) ----
N = 4
C = 128
H = 64
W = 64
KS = 5
PAD = 2
SCALE = 2
WP = W + KS - 1          # 68 contraction width per feature row
NB = SCALE * W           # 128 upsampled cols per hup row
RUN = 4 * KS             # 20 elems per diagonal run (w,b,a interleaved)
REG = 2 * NB + 32        # 288 per-band region: 16 pad | 256 data | 16 pad
BW = KS * REG            # 1440 band buffer free width
NH = H // 2              # 32 low-res rows per core
NROWS = NH + 4           # 36 feature rows per shard (halo zero-padded)
N_BBUF = 8
OBATCH = 8               # jobs per output DMA

F32 = mybir.dt.float32
F32R = mybir.dt.float32r

_PROG_CACHE: dict = {}


def _device_body(tc, ctx, out_ap, ft_ap, msk3_ap):
    nc = tc.nc
    if True:
        sb = ctx.enter_context(tc.tile_pool(name="sb", bufs=1))
        psum = ctx.enter_context(tc.tile_pool(name="ps", bufs=4, space="PSUM"))
        obp = ctx.enter_context(tc.tile_pool(name="ob", bufs=3))

        # chunked input loads, spread across both HWDGE rings so job 0's
        # data lands early and loads overlap compute
        ft = sb.tile([WP, NROWS * C], F32)
        mst = sb.tile([WP, NH * KS * RUN], F32)
        mstap = mst[:]
        MCH = 4 * KS * RUN                     # mask cols per 4-job chunk
        n_mch = NH // 4
        ft_bounds = [0, 11, 20, 29, NROWS]     # rows: jobs 0-6 / -15 / -24 / -31
        mch, fch = 0, 0
        order = [("m", 0), ("f", 0), ("m", 1), ("f", 1), ("m", 2), ("f", 2),
                 ("m", 3), ("f", 3)] + [("m", g) for g in range(4, n_mch)]
        for k2, (kind, g) in enumerate(order):
            eng = (nc.sync, nc.scalar, nc.gpsimd)[min(k2, 2)]
            if kind == "m":
                eng.dma_start(
                    mst[:, g * MCH : (g + 1) * MCH],
                    msk3_ap[:, g * MCH : (g + 1) * MCH],
                )
                mch += 1
            else:
                lo, hi = ft_bounds[g] * C, ft_bounds[g + 1] * C
                eng.dma_start(
                    ft[:, lo:hi].bitcast(F32R), ft_ap[:, lo:hi].bitcast(F32R)
                )
                fch += 1

        # persistent band buffers, memset once (static sparsity pattern)
        bbufs = []
        for q in range(N_BBUF):
            b = sb.tile([WP, BW], F32, tag=f"bbuf{q}")
            nc.vector.memset(b[:], 0.0)
            bbufs.append(b)

        ob4 = None
        for hl in range(NH):
            bap = bbufs[hl % N_BBUF][:]
            # SBUF->SBUF diagonal scatter: all 5 bands' runs for this job.
            # dest: [w' (+1 part,+4 col)][i: region][t: run]
            dst = AP(bap.tensor, bap.offset, [[BW + 4, WP], [REG, KS], [1, RUN]])
            src = AP(
                mstap.tensor,
                mstap.offset + hl * KS * RUN,
                [[NH * KS * RUN, WP], [RUN, KS], [1, RUN]],
            )
            if hl % 2 == 0:
                eng = nc.sync if (hl // 2) % 2 == 0 else nc.scalar
            else:
                eng = nc.gpsimd
            eng.dma_start(dst.bitcast(F32R), src.bitcast(F32R))

            ps = psum.tile([C, 2 * NB], F32)
            for i in range(KS):
                lhsT = ft[:, (hl + i) * C : (hl + i + 1) * C].bitcast(F32R)
                rhs = AP(
                    bap.tensor,
                    bap.offset + i * REG + 16,
                    [[BW, WP], [1, 2], [2, NB]],
                ).bitcast(F32R)
                nc.tensor.matmul(ps[:], lhsT, rhs, start=(i == 0), stop=(i == 4))

            if hl % OBATCH == 0:
                ob4 = obp.tile([C, OBATCH * 2 * NB], F32)
            sl = ob4[:, (hl % OBATCH) * 2 * NB : (hl % OBATCH + 1) * 2 * NB]
            if hl % 2 == 0:
                nc.scalar.copy(sl, ps[:])
            else:
                nc.vector.tensor_copy(sl, ps[:])
            if hl == NH - 5:
                g = hl - (OBATCH - 5)
                nc.gpsimd.dma_start(
                    out_ap[:, 2 * g : 2 * g + 8, :], ob4[:, : 4 * 2 * NB]
                )
            elif hl == NH - 1:
                nc.scalar.dma_start(
                    out_ap[:, 2 * (NH - 4) : 2 * NH, :], ob4[:, 4 * 2 * NB :]
                )
            elif hl % OBATCH == OBATCH - 1:
                g = hl - (OBATCH - 1)
                nc.gpsimd.dma_start(
                    out_ap[:, 2 * g : 2 * g + 2 * OBATCH, :], ob4[:]
                )


def _build_program():
    nc = bacc.Bacc(
        "TRN2", debug=False, enable_asserts=False, target_bir_lowering=False
    )
    ft_t = nc.dram_tensor("ft", [WP, NROWS * C], F32, kind="ExternalInput")
    msk_t = nc.dram_tensor("msk3", [WP, NH * KS * RUN], F32, kind="ExternalInput")
    out_t = nc.dram_tensor("out", [C, 2 * NH, NB], F32, kind="ExternalOutput")

    with tile.TileContext(nc) as tc, ExitStack() as ctx:
        _device_body(tc, ctx, out_t.ap(), ft_t.ap(), msk_t.ap())
    nc.compile()
    return nc


def _prep_ft(feat_n: np.ndarray, h0: int) -> np.ndarray:
    """[C,H,W] -> fT[w'', r, c] with r over [h0-2, h0+NH+2), zero-padded."""
    ft = np.zeros((WP, NROWS, C), np.float32)
    r_lo, r_hi = h0 - 2, h0 + NH + 2
    s_lo, s_hi = max(r_lo, 0), min(r_hi, H)
    # f[c, r, w] -> [w, r, c]
    ft[PAD : PAD + W, s_lo - r_lo : s_hi - r_lo, :] = feat_n[:, s_lo:s_hi, :].transpose(
        2, 1, 0
    )
    return np.ascontiguousarray(ft.reshape(WP, NROWS * C))


def _prep_msk3(masks_n: np.ndarray) -> np.ndarray:
    """[25, 2H, 2W] -> msk3[w', h, i, t20]  [WP, H, KS, RUN]
    t20 = (w - (w'-4))*4 + b*2 + a; value = masks[5i + (4 - t20//4), 2h+a, 2w+b]
    """
    tt = np.arange(RUN)
    wpp = np.arange(WP)
    dw = tt // 4
    b = (tt % 4) // 2
    a = tt % 2
    j = 4 - dw
    wup = 2 * (wpp[:, None] - 4 + dw[None, :]) + b[None, :]
    wup_c = np.clip(wup, 0, 2 * W - 1)                     # [WP, RUN]
    i_ar = np.arange(KS)
    k_full = 5 * i_ar[:, None] + j[None, :]                # [KS, RUN]
    hh = np.arange(H)
    hup = 2 * hh[:, None] + a[None, :]                     # [H, RUN]
    out = masks_n[
        k_full[None, None, :, :],
        hup[None, :, None, :],
        wup_c[:, None, None, :],
    ]  # [WP, H, KS, RUN]
    return np.ascontiguousarray(out.astype(np.float32))


def kernel(features: np.ndarray, masks: np.ndarray, _perf: dict | None = None):
    features = np.asarray(features, dtype=np.float32)
    masks = np.asarray(masks, dtype=np.float32)

    if "nc" not in _PROG_CACHE:
        _PROG_CACHE["nc"] = _build_program()
    nc = _PROG_CACHE["nc"]

    in_maps = []
    for core in range(8):
        n, half = divmod(core, 2)
        h0 = NH * half
        ft_sh = _prep_ft(features[n], h0)
        msk3 = _prep_msk3(masks[n])[:, h0 : h0 + NH]  # [WP, NH, KS, RUN]
        in_maps.append(
            {
                "ft": ft_sh,
                "msk3": np.ascontiguousarray(msk3.reshape(WP, NH * KS * RUN)),
            }
        )

    trace = bool(_perf is not None and _perf.get("trace"))
    res = run_bass_kernel_spmd(
        nc, in_maps, core_ids=list(range(8)), trace=trace,
        **({} if not trace else {"trace_cores": [0]}),
    )
    if _perf is not None:
        _perf["exec_time_ns"] = res.exec_time_ns
        _perf["trace"] = res.instructions_and_trace

    out = np.empty((N, C, SCALE * H, SCALE * W), np.float32)
    for core in range(8):
        n, half = divmod(core, 2)
        out[n, :, 64 * half : 64 * half + 64, :] = res.results[core]["out"]
    return out



# revision 3
# speedup vs baseline: 1.3379x; 1.3379x over previous
"""CARAFE content-aware upsampling on 8 Trainium2 NeuronCores (Bass/Tile).

Problem: features (4,128,64,64) f32, masks (4,25,128,128) f32
         -> out (4,128,128,128) f32
out[n,c,2h+a,2w+b] = sum_{i,j in 5x5} f[n,c,h+i-2,w+j-2] * m[n,5i+j,2h+a,2w+b]

Strategy (per core = one (n, h-half) shard), v2:
  All device data in bf16 (PSUM accumulates f32; host up-converts the
  bf16 output).  For each low-res row h we compute out[c, (a, wup)]
  (two upsampled rows, 256 cols) as 5 PSUM-accumulated matmuls, one per
  kernel-row i:  out += fT_row(h+i-2).T @ B_i  where fT_row is the
  W-padded transposed feature row [w''(68), c(128)] and B_i
  [w''(68), 256] is a banded matrix holding the masks on diagonals.
  Bands are materialized by a per-job DIRECT HBM->SBUF diagonal-scatter
  DMA (dest AP steps +1 partition +4 elements) from the host-rearranged
  mask tensor; HBM-sourced packets address-interleave across all 16 DMA
  engines (SBUF->SBUF packets would pin to the 4 HWDGE ring engines),
  and the two HWDGE rings (sync/scalar) keep descriptor generation off
  the software DGE (which emits ~3x extra 4B packets per data packet).
  The band sparsity pattern is static, so the zero background is memset
  once and the 20-element runs are overwritten in place; run overrun at
  the edges lands in 16-element pad gaps between the five band regions.
  Bulk I/O (feature rows in, bf16 output out) rides the gpsimd SWDGE
  queue in multi-KB packets.
"""
import sys

if "/opt/trn_rl_repo" not in sys.path:
    sys.path.insert(0, "/opt/trn_rl_repo")

from contextlib import ExitStack

import ml_dtypes
import numpy as np

import concourse.tile as tile
from concourse import bacc, mybir
from concourse.ap import AP
from concourse.bass_utils import run_bass_kernel_spmd

# ---- problem constants (hardcoded per harness contract) ----
N = 4
C = 128
H = 64
W = 64
KS = 5
PAD = 2
SCALE = 2
WP = W + KS - 1          # 68 contraction width per feature row
NB = SCALE * W           # 128 upsampled cols per hup row
RUN = 4 * KS             # 20 elems per diagonal run (w,b,a interleaved)
REG = 2 * NB + 32        # 288 per-band region: 16 pad | 256 data | 16 pad
BW = KS * REG            # 1440 band buffer free width
NH = H // 2              # 32 low-res rows per core
NROWS = NH + 4           # 36 feature rows per shard (halo zero-padded)
MCOL = KS * RUN          # 100 mask elems per (partition, job)
N_BBUF = 8
OBATCH = 8               # jobs per output DMA

F32 = mybir.dt.float32
BF16 = mybir.dt.bfloat16
BF16NP = ml_dtypes.bfloat16

_PROG_CACHE: dict = {}


def _device_body(tc, ctx, out_ap, ft_ap, msk_ap):
    nc = tc.nc
    sb = ctx.enter_context(tc.tile_pool(name="sb", bufs=1))
    psum = ctx.enter_context(tc.tile_pool(name="ps", bufs=4, space="PSUM"))
    obp = ctx.enter_context(tc.tile_pool(name="ob", bufs=3))

    # feature rows, chunked so job 0's rows land early; big HBM->SBUF
    # packets spread across all DMA engines.  gpsimd = SWDGE queue.
    ft = sb.tile([WP, NROWS * C], BF16)
    ft_bounds = [0, 11, 20, 29, NROWS]
    for g in range(4):
        lo, hi = ft_bounds[g] * C, ft_bounds[g + 1] * C
        nc.gpsimd.dma_start(ft[:, lo:hi], ft_ap[:, lo:hi])

    # persistent band buffers, memset once (static sparsity pattern).
    # Spread the memsets over three engines so buffer 0/1 are ready fast.
    bbufs = []
    for q in range(N_BBUF):
        b = sb.tile([WP, BW], BF16, tag=f"bbuf{q}")
        eng = nc.vector if q % 2 == 0 else nc.gpsimd
        eng.memset(b[:], 0.0)
        bbufs.append(b)

    def scatter(hl):
        """Direct HBM->SBUF diagonal scatter of job hl's mask runs."""
        bap = bbufs[hl % N_BBUF][:]
        dst = AP(bap.tensor, bap.offset, [[BW + 4, WP], [REG, KS], [1, RUN]])
        src = AP(
            msk_ap.tensor,
            msk_ap.offset + hl * MCOL,
            [[NH * MCOL, WP], [RUN, KS], [1, RUN]],
        )
        eng = nc.sync if hl % 2 == 0 else nc.scalar
        eng.dma_start(dst, src)

    for hl in range(N_BBUF):
        scatter(hl)

    ob4 = None
    for hl in range(NH):
        bap = bbufs[hl % N_BBUF][:]
        ps = psum.tile([C, 2 * NB], F32)
        for i in range(KS):
            lhsT = ft[:, (hl + i) * C : (hl + i + 1) * C]
            rhs = AP(
                bap.tensor,
                bap.offset + i * REG + 16,
                [[BW, WP], [1, 2], [2, NB]],
            )
            nc.tensor.matmul(ps[:], lhsT, rhs, start=(i == 0), stop=(i == 4))

        # refill this band buffer for job hl+N_BBUF (band reads done)
        if hl + N_BBUF < NH:
            scatter(hl + N_BBUF)

        if hl % OBATCH == 0:
            ob4 = obp.tile([C, OBATCH * 2 * NB], BF16)
        sl = ob4[:, (hl % OBATCH) * 2 * NB : (hl % OBATCH + 1) * 2 * NB]
        if hl % 2 == 0:
            nc.scalar.copy(sl, ps[:])
        else:
            nc.vector.tensor_copy(sl, ps[:])
        if hl == NH - 5:
            g = hl - (OBATCH - 5)
            nc.gpsimd.dma_start(
                out_ap[:, 2 * g : 2 * g + 8, :], ob4[:, : 4 * 2 * NB]
            )
        elif hl == NH - 1:
            nc.gpsimd.dma_start(
                out_ap[:, 2 * (NH - 4) : 2 * NH, :], ob4[:, 4 * 2 * NB :]
            )
        elif hl % OBATCH == OBATCH - 1:
            g = hl - (OBATCH - 1)
            nc.gpsimd.dma_start(
                out_ap[:, 2 * g : 2 * g + 2 * OBATCH, :], ob4[:]
            )


def _build_program():
    nc = bacc.Bacc(
        "TRN2", debug=False, enable_asserts=False, target_bir_lowering=False
    )
    ft_t = nc.dram_tensor("ft", [WP, NROWS * C], BF16, kind="ExternalInput")
    msk_t = nc.dram_tensor("msk3", [WP, NH * MCOL], BF16, kind="ExternalInput")
    out_t = nc.dram_tensor("out", [C, 2 * NH, NB], BF16, kind="ExternalOutput")

    with tile.TileContext(nc) as tc, ExitStack() as ctx:
        _device_body(tc, ctx, out_t.ap(), ft_t.ap(), msk_t.ap())
    nc.compile()
    return nc


def _prep_ft(feat_n: np.ndarray, h0: int) -> np.ndarray:
    """[C,H,W] -> fT[w'', r, c] with r over [h0-2, h0+NH+2), zero-padded."""
    ft = np.zeros((WP, NROWS, C), BF16NP)
    r_lo, r_hi = h0 - 2, h0 + NH + 2
    s_lo, s_hi = max(r_lo, 0), min(r_hi, H)
    ft[PAD : PAD + W, s_lo - r_lo : s_hi - r_lo, :] = (
        feat_n[:, s_lo:s_hi, :].transpose(2, 1, 0).astype(BF16NP)
    )
    return np.ascontiguousarray(ft.reshape(WP, NROWS * C))


def _prep_msk3(masks_n: np.ndarray) -> np.ndarray:
    """[25, 2H, 2W] -> msk3[w', h, i, t20]  [WP, H, KS, RUN]
    t20 = (w - (w'-4))*4 + b*2 + a; value = masks[5i + (4 - t20//4), 2h+a, 2w+b]
    """
    tt = np.arange(RUN)
    wpp = np.arange(WP)
    dw = tt // 4
    b = (tt % 4) // 2
    a = tt % 2
    j = 4 - dw
    wup = 2 * (wpp[:, None] - 4 + dw[None, :]) + b[None, :]
    wup_c = np.clip(wup, 0, 2 * W - 1)                     # [WP, RUN]
    i_ar = np.arange(KS)
    k_full = 5 * i_ar[:, None] + j[None, :]                # [KS, RUN]
    hh = np.arange(H)
    hup = 2 * hh[:, None] + a[None, :]                     # [H, RUN]
    out = masks_n[
        k_full[None, None, :, :],
        hup[None, :, None, :],
        wup_c[:, None, None, :],
    ]  # [WP, H, KS, RUN]
    return np.ascontiguousarray(out.astype(BF16NP))


def kernel(features: np.ndarray, masks: np.ndarray, _perf: dict | None = None):
    features = np.asarray(features, dtype=np.float32)
    masks = np.asarray(masks, dtype=np.float32)

    if "nc" not in _PROG_CACHE:
        _PROG_CACHE["nc"] = _build_program()
    nc = _PROG_CACHE["nc"]

    in_maps = []
    for core in range(8):
        n, half = divmod(core, 2)
        h0 = NH * half
        ft_sh = _prep_ft(features[n], h0)
        msk3 = _prep_msk3(masks[n])[:, h0 : h0 + NH]  # [WP, NH, KS, RUN]
        in_maps.append(
            {
                "ft": ft_sh,
                "msk3": np.ascontiguousarray(msk3.reshape(WP, NH * MCOL)),
            }
        )

    trace = bool(_perf is not None and _perf.get("trace"))
    res = run_bass_kernel_spmd(
        nc, in_maps, core_ids=list(range(8)), trace=trace,
        **({} if not trace else {"trace_cores": [0]}),
    )
    if _perf is not None:
        _perf["exec_time_ns"] = res.exec_time_ns
        _perf["trace"] = res.instructions_and_trace

    out = np.empty((N, C, SCALE * H, SCALE * W), np.float32)
    for core in range(8):
        n, half = divmod(core, 2)
        out[n, :, 64 * half : 64 * half + 64, :] = np.asarray(
            res.results[core]["out"], dtype=np.float32
        )
    return out


# revision 5
# speedup vs baseline: 1.3530x; 1.0112x over previous
"""CARAFE content-aware upsampling on 8 Trainium2 NeuronCores (Bass/Tile).

Problem: features (4,128,64,64) f32, masks (4,25,128,128) f32
         -> out (4,128,128,128) f32
out[n,c,2h+a,2w+b] = sum_{i,j in 5x5} f[n,c,h+i-2,w+j-2] * m[n,5i+j,2h+a,2w+b]

Strategy (per core = one (n, h-half) shard), v3:
  All device data in bf16 (PSUM accumulates f32; host up-converts and
  reorders the bf16 output).  For each low-res row h we compute
  out[c, (w,b,a)] (256 cols, both upsampled sub-rows) as 5
  PSUM-accumulated matmuls, one per kernel-row i:
     out += fT_row(h+i-2).T @ B_i
  where fT_row is the W-padded transposed feature row [w''(68), c(128)]
  and B_i [w''(68), 256] is a banded matrix holding the masks on
  diagonals.  The moving operand is streamed STRIDE-1 (col order
  (w,b,a)), which doubles PE ifmap fetch throughput vs the stride-2
  (a,w,b) order; the host untangles the resulting [*,w,b,a] output
  layout after upconversion.
  Bands are materialized by per-job DIRECT HBM->SBUF diagonal-scatter
  DMAs (dest AP steps +1 partition +4 elements) from host-rearranged
  masks, round-robined over the two HWDGE rings (sync/scalar; their 4
  ring engines sustain ~10ns/packet) and the gpsimd SWDGE queue (which
  spreads packets over all 16 DMA engines at the cost of software
  descriptor generation).  The band sparsity pattern is static: zero
  background memset once, 20-element runs overwritten in place, edge
  overrun lands in 16-element pad gaps between band regions.
  Bulk I/O (feature rows, bf16 output) rides gpsimd in multi-KB
  packets; the output is drained in shrinking batches (8,8,8,4,2,2
  jobs) so the post-matmul tail is short.
"""
import sys

if "/opt/trn_rl_repo" not in sys.path:
    sys.path.insert(0, "/opt/trn_rl_repo")

from contextlib import ExitStack

import ml_dtypes
import numpy as np

import concourse.tile as tile
from concourse import bacc, mybir
from concourse.ap import AP
from concourse.bass_utils import run_bass_kernel_spmd

# ---- problem constants (hardcoded per harness contract) ----
N = 4
C = 128
H = 64
W = 64
KS = 5
PAD = 2
SCALE = 2
WP = W + KS - 1          # 68 contraction width per feature row
NB = SCALE * W           # 128 upsampled cols per hup row
RUN = 4 * KS             # 20 elems per diagonal run (w,b,a interleaved)
REG = 2 * NB + 32        # 288 per-band region: 16 pad | 256 data | 16 pad
BW = KS * REG            # 1440 band buffer free width
NH = H // 2              # 32 low-res rows per core
NROWS = NH + 4           # 36 feature rows per shard (halo zero-padded)
MCOL = KS * RUN          # 100 mask elems per (partition, job)
N_BBUF = 8
OB_ENDS = [8, 16, 24, 28, 30, 32]   # output batch boundaries (jobs)

F32 = mybir.dt.float32
BF16 = mybir.dt.bfloat16
BF16NP = ml_dtypes.bfloat16

_PROG_CACHE: dict = {}


def _device_body(tc, ctx, out_ap, ft_ap, msk_ap):
    nc = tc.nc
    sb = ctx.enter_context(tc.tile_pool(name="sb", bufs=1))
    psum = ctx.enter_context(tc.tile_pool(name="ps", bufs=4, space="PSUM"))
    obp = ctx.enter_context(tc.tile_pool(name="ob", bufs=3))

    ft = sb.tile([WP, NROWS * C], BF16)
    bbufs = []
    for q in range(N_BBUF):
        b = sb.tile([WP, BW], BF16, tag=f"bbuf{q}")
        bbufs.append(b)

    def scatter(hl):
        """Direct HBM->SBUF diagonal scatter of job hl's mask runs."""
        bap = bbufs[hl % N_BBUF][:]
        dst = AP(bap.tensor, bap.offset, [[BW + 4, WP], [REG, KS], [1, RUN]])
        src = AP(
            msk_ap.tensor,
            msk_ap.offset + hl * MCOL,
            [[NH * MCOL, WP], [RUN, KS], [1, RUN]],
        )
        eng = (nc.sync, nc.scalar, nc.gpsimd)[hl % 3]
        eng.dma_start(dst, src)

    # startup: interleave band-buffer memsets, first scatters, and the
    # feature-row loads so job 0 can start ~3us in.  vector zeroes the
    # even buffers, gpsimd the odd ones; scatter j only needs buffer
    # j%8, and the tile framework serializes per-buffer.
    nc.vector.memset(bbufs[0][:], 0.0)
    nc.gpsimd.memset(bbufs[1][:], 0.0)
    ft_bounds = [0, 11, 20, 29, NROWS]
    lo, hi = 0, ft_bounds[1] * C
    nc.gpsimd.dma_start(ft[:, lo:hi], ft_ap[:, lo:hi])   # rows for jobs 0-6
    scatter(0)
    scatter(1)
    nc.vector.memset(bbufs[2][:], 0.0)
    nc.gpsimd.memset(bbufs[3][:], 0.0)
    scatter(2)
    for g in range(1, 4):
        lo, hi = ft_bounds[g] * C, ft_bounds[g + 1] * C
        nc.gpsimd.dma_start(ft[:, lo:hi], ft_ap[:, lo:hi])
    scatter(3)
    nc.vector.memset(bbufs[4][:], 0.0)
    nc.gpsimd.memset(bbufs[5][:], 0.0)
    scatter(4)
    scatter(5)
    nc.vector.memset(bbufs[6][:], 0.0)
    nc.gpsimd.memset(bbufs[7][:], 0.0)
    scatter(6)
    scatter(7)

    ob4 = None
    ob_lo = 0
    for hl in range(NH):
        bap = bbufs[hl % N_BBUF][:]
        ps = psum.tile([C, 2 * NB], F32)
        for i in range(KS):
            lhsT = ft[:, (hl + i) * C : (hl + i + 1) * C]
            rhs = AP(bap.tensor, bap.offset + i * REG + 16, [[BW, WP], [1, 2 * NB]])
            nc.tensor.matmul(ps[:], lhsT, rhs, start=(i == 0), stop=(i == 4))

        # refill this band buffer for job hl+N_BBUF (band reads done)
        if hl + N_BBUF < NH:
            scatter(hl + N_BBUF)

        if hl == ob_lo:
            ob_hi = min(e for e in OB_ENDS if e > hl)
            ob4 = obp.tile([C, (ob_hi - ob_lo) * 2 * NB], BF16)
        sl = ob4[:, (hl - ob_lo) * 2 * NB : (hl - ob_lo + 1) * 2 * NB]
        nc.vector.tensor_copy(sl, ps[:])
        if hl == ob_hi - 1:
            nc.gpsimd.dma_start(out_ap[:, ob_lo : ob_hi, :], ob4[:])
            ob_lo = ob_hi


def _build_program():
    nc = bacc.Bacc(
        "TRN2", debug=False, enable_asserts=False, target_bir_lowering=False
    )
    ft_t = nc.dram_tensor("ft", [WP, NROWS * C], BF16, kind="ExternalInput")
    msk_t = nc.dram_tensor("msk3", [WP, NH * MCOL], BF16, kind="ExternalInput")
    # device output layout: [c, h, (w,b,a)] -- host reorders to (a,w,b)
    out_t = nc.dram_tensor("out", [C, NH, 2 * NB], BF16, kind="ExternalOutput")

    with tile.TileContext(nc) as tc, ExitStack() as ctx:
        _device_body(tc, ctx, out_t.ap(), ft_t.ap(), msk_t.ap())
    nc.compile()
    return nc


def _prep_ft(feat_n: np.ndarray, h0: int) -> np.ndarray:
    """[C,H,W] -> fT[w'', r, c] with r over [h0-2, h0+NH+2), zero-padded."""
    ft = np.zeros((WP, NROWS, C), BF16NP)
    r_lo, r_hi = h0 - 2, h0 + NH + 2
    s_lo, s_hi = max(r_lo, 0), min(r_hi, H)
    ft[PAD : PAD + W, s_lo - r_lo : s_hi - r_lo, :] = (
        feat_n[:, s_lo:s_hi, :].transpose(2, 1, 0).astype(BF16NP)
    )
    return np.ascontiguousarray(ft.reshape(WP, NROWS * C))


def _prep_msk3(masks_n: np.ndarray) -> np.ndarray:
    """[25, 2H, 2W] -> msk3[w', h, i, t20]  [WP, H, KS, RUN]
    t20 = (w - (w'-4))*4 + b*2 + a; value = masks[5i + (4 - t20//4), 2h+a, 2w+b]
    """
    tt = np.arange(RUN)
    wpp = np.arange(WP)
    dw = tt // 4
    b = (tt % 4) // 2
    a = tt % 2
    j = 4 - dw
    wup = 2 * (wpp[:, None] - 4 + dw[None, :]) + b[None, :]
    wup_c = np.clip(wup, 0, 2 * W - 1)                     # [WP, RUN]
    i_ar = np.arange(KS)
    k_full = 5 * i_ar[:, None] + j[None, :]                # [KS, RUN]
    hh = np.arange(H)
    hup = 2 * hh[:, None] + a[None, :]                     # [H, RUN]
    out = masks_n[
        k_full[None, None, :, :],
        hup[None, :, None, :],
        wup_c[:, None, None, :],
    ]  # [WP, H, KS, RUN]
    return np.ascontiguousarray(out.astype(BF16NP))


def kernel(features: np.ndarray, masks: np.ndarray, _perf: dict | None = None):
    features = np.asarray(features, dtype=np.float32)
    masks = np.asarray(masks, dtype=np.float32)

    if "nc" not in _PROG_CACHE:
        _PROG_CACHE["nc"] = _build_program()
    nc = _PROG_CACHE["nc"]

    in_maps = []
    for core in range(8):
        n, half = divmod(core, 2)
        h0 = NH * half
        ft_sh = _prep_ft(features[n], h0)
        msk3 = _prep_msk3(masks[n])[:, h0 : h0 + NH]  # [WP, NH, KS, RUN]
        in_maps.append(
            {
                "ft": ft_sh,
                "msk3": np.ascontiguousarray(msk3.reshape(WP, NH * MCOL)),
            }
        )

    trace = bool(_perf is not None and _perf.get("trace"))
    res = run_bass_kernel_spmd(
        nc, in_maps, core_ids=list(range(8)), trace=trace,
        **({} if not trace else {"trace_cores": [0]}),
    )
    if _perf is not None:
        _perf["exec_time_ns"] = res.exec_time_ns
        _perf["trace"] = res.instructions_and_trace

    out = np.empty((N, C, SCALE * H, SCALE * W), np.float32)
    for core in range(8):
        n, half = divmod(core, 2)
        dev = np.asarray(res.results[core]["out"], dtype=np.float32)
        # [c, h, w, b, a] -> [c, (h,a), (w,b)]
        dev = dev.reshape(C, NH, W, 2, 2).transpose(0, 1, 4, 2, 3)
        out[n, :, 64 * half : 64 * half + 64, :] = dev.reshape(C, 2 * NH, 2 * W)
    return out
